# revision 5
# baseline (speedup 1.0000x reference)
"""BayesianLinear (y = x @ (mu + softplus(rho) * eps).T + bias) on 8 TRN2 cores.

Column-parallel sharding: each core owns OUT_F/8 = 512 output features.
x is replicated (host pre-cast to bf16 so the XBAR DMA-transpose path can
produce x^T tiles directly); weight/bias params are sharded along
out_features and stay fp32 for the on-device weight construction.

Per-core pipeline:
  1. bias row  = bias_mu + softplus(bias_rho) * bias_eps        (fp32 -> bf16)
  2. W shard   = weight_mu + softplus(weight_rho) * weight_eps  (fp32, ACT+DVE)
     cast to bf16, bounce through DRAM scratch, transposed-load as
     W^T [128, IN_F/128, 512] (K on partitions).
  3. For each 128-row batch tile: transposed-load x^T [128, IN_F/128, 128],
     accumulate 32 bf16 matmuls into PSUM [128, 512] fp32, add bias via a
     K=1 matmul against a ones row, evict PSUM->SBUF on DVE, DMA out.
"""

import numpy as np
import ml_dtypes

import concourse.bacc as bacc
import concourse.mybir as mybir
import concourse.tile as tile
from concourse.bass_utils import run_bass_kernel_spmd

BATCH = 8192
IN_F = 4096
OUT_F = 4096
N_CORES = 8
P = 128

_NC_CACHE = {}


def build_nc(batch=BATCH, in_f=IN_F, o_sh=OUT_F // N_CORES):
    KB = in_f // P  # k-blocks of 128 along the contraction dim
    BT = batch // P  # 128-row output tiles
    OB = o_sh // P  # 128-row weight-construction blocks
    IC = 1024 if in_f % 1024 == 0 else in_f  # weight-construction column chunk
    NI = in_f // IC

    nc = bacc.Bacc(
        "TRN2",
        target_bir_lowering=False,
        debug=False,
        enable_asserts=False,
        num_devices=N_CORES,
    )
    bf16 = mybir.dt.bfloat16
    f32 = mybir.dt.float32

    x = nc.declare_dram_parameter("x_bf16", [batch, in_f], bf16, isOutput=False)
    wmu = nc.declare_dram_parameter("weight_mu", [o_sh, in_f], f32, isOutput=False)
    wrho = nc.declare_dram_parameter("weight_rho", [o_sh, in_f], f32, isOutput=False)
    weps = nc.declare_dram_parameter("weight_eps", [o_sh, in_f], f32, isOutput=False)
    bmu = nc.declare_dram_parameter("bias_mu", [1, o_sh], f32, isOutput=False)
    brho = nc.declare_dram_parameter("bias_rho", [1, o_sh], f32, isOutput=False)
    beps = nc.declare_dram_parameter("bias_eps", [1, o_sh], f32, isOutput=False)
    y = nc.declare_dram_parameter("y", [batch, o_sh], f32, isOutput=True)
    w_scr = nc.dram_tensor("w_scr", [o_sh, in_f], bf16)

    # No Softplus LUT on TRN2; Exp and Ln share one table, so
    # softplus(x) = Ln(1 + Exp(x)) costs two ACT passes and one table load.
    act_exp = mybir.ActivationFunctionType.Exp
    act_ln = mybir.ActivationFunctionType.Ln

    with tile.TileContext(nc) as tc:
        with (
            tc.tile_pool(name="const", bufs=1) as const,
            tc.tile_pool(name="wcons", bufs=2) as wcons,
            tc.tile_pool(name="xin", bufs=4) as xin,
            tc.tile_pool(name="yout", bufs=4) as yout,
            tc.tile_pool(name="psum", bufs=4, space="PSUM") as psum_pool,
        ):
            # ---- bias row: bf16 [1, o_sh], plus a ones row for the K=1 matmul
            b_mu = const.tile([1, o_sh], f32, tag="b_mu")
            b_rho = const.tile([1, o_sh], f32, tag="b_rho")
            b_eps = const.tile([1, o_sh], f32, tag="b_eps")
            nc.sync.dma_start(out=b_mu[:], in_=bmu[:])
            nc.sync.dma_start(out=b_rho[:], in_=brho[:])
            nc.sync.dma_start(out=b_eps[:], in_=beps[:])
            b_sp = const.tile([1, o_sh], f32, tag="b_sp")
            nc.scalar.activation(b_sp[:], b_rho[:], act_exp)
            nc.scalar.activation(b_sp[:], b_sp[:], act_ln, bias=1.0)
            nc.vector.tensor_mul(out=b_sp[:], in0=b_sp[:], in1=b_eps[:])
            bias_bf = const.tile([1, o_sh], bf16, tag="bias_bf")
            nc.vector.tensor_add(out=bias_bf[:], in0=b_sp[:], in1=b_mu[:])
            ones = const.tile([1, P], bf16, tag="ones")
            nc.vector.memset(ones[:], 1.0)

            # ---- W shard: construct fp32, cast bf16, bounce via DRAM scratch
            for ob in range(OB):
                for ic in range(NI):
                    cs = slice(ic * IC, (ic + 1) * IC)
                    rs = slice(ob * P, (ob + 1) * P)
                    mu_t = wcons.tile([P, IC], f32, tag="mu")
                    rho_t = wcons.tile([P, IC], f32, tag="rho")
                    eps_t = wcons.tile([P, IC], f32, tag="eps")
                    nc.sync.dma_start(out=mu_t[:], in_=wmu[rs, cs])
                    nc.sync.dma_start(out=rho_t[:], in_=wrho[rs, cs])
                    nc.sync.dma_start(out=eps_t[:], in_=weps[rs, cs])
                    sp_t = wcons.tile([P, IC], f32, tag="sp")
                    nc.scalar.activation(sp_t[:], rho_t[:], act_exp)
                    nc.scalar.activation(sp_t[:], sp_t[:], act_ln, bias=1.0)
                    nc.vector.tensor_mul(out=sp_t[:], in0=sp_t[:], in1=eps_t[:])
                    w_bf = wcons.tile([P, IC], bf16, tag="w_bf")
                    nc.vector.tensor_add(out=w_bf[:], in0=sp_t[:], in1=mu_t[:])
                    nc.sync.dma_start(out=w_scr[rs, cs], in_=w_bf[:])

            # ---- W^T: one transposed load [o_sh, in_f] -> [128, KB, o_sh]
            WT = const.tile([P, KB, o_sh], bf16, tag="WT")
            nc.sync.dma_start_transpose(WT[:], w_scr[:, :])

            # ---- main loop over 128-row output tiles
            for bt in range(BT):
                rs = slice(bt * P, (bt + 1) * P)
                xT = xin.tile([P, KB, P], bf16, tag="xT")
                nc.sync.dma_start_transpose(xT[:], x[rs, :])
                ps = psum_pool.tile([P, o_sh], f32, tag="ps")
                for k in range(KB):
                    nc.tensor.matmul(
                        ps[:],
                        lhsT=xT[:, k, :],
                        rhs=WT[:, k, :],
                        start=(k == 0),
                        stop=False,
                    )
                nc.tensor.matmul(
                    ps[:], lhsT=ones[:], rhs=bias_bf[:], start=False, stop=True
                )
                y_sb = yout.tile([P, o_sh], f32, tag="y_sb")
                nc.vector.tensor_copy(out=y_sb[:], in_=ps[:])
                nc.sync.dma_start(out=y[rs, :], in_=y_sb[:])

    # Skip bacc's pre-placed InstLoadActFuncSet: on large graphs walrus's
    # parallel-pass fork can separate the hoisted load from its activations
    # ("No Act func set exist for this instruction"); walrus's own lower_act
    # placement handles forked subgraphs correctly.
    nc.insert_act_table_loads = lambda: None
    nc.compile()
    return nc


def kernel(x, weight_mu, weight_rho, bias_mu, bias_rho, weight_eps, bias_eps):
    o_sh = OUT_F // N_CORES
    key = (x.shape, o_sh)
    if key not in _NC_CACHE:
        _NC_CACHE[key] = build_nc(x.shape[0], x.shape[1], o_sh)
    nc = _NC_CACHE[key]

    x_bf16 = np.asarray(x, dtype=np.float32).astype(ml_dtypes.bfloat16)
    wmu = np.ascontiguousarray(np.asarray(weight_mu, dtype=np.float32))
    wrho = np.ascontiguousarray(np.asarray(weight_rho, dtype=np.float32))
    weps = np.ascontiguousarray(np.asarray(weight_eps, dtype=np.float32))
    bmu = np.asarray(bias_mu, dtype=np.float32).reshape(1, -1)
    brho = np.asarray(bias_rho, dtype=np.float32).reshape(1, -1)
    beps = np.asarray(bias_eps, dtype=np.float32).reshape(1, -1)

    in_maps = []
    for c in range(N_CORES):
        rs = slice(c * o_sh, (c + 1) * o_sh)
        in_maps.append(
            {
                "x_bf16": x_bf16,
                "weight_mu": np.ascontiguousarray(wmu[rs]),
                "weight_rho": np.ascontiguousarray(wrho[rs]),
                "weight_eps": np.ascontiguousarray(weps[rs]),
                "bias_mu": np.ascontiguousarray(bmu[:, rs]),
                "bias_rho": np.ascontiguousarray(brho[:, rs]),
                "bias_eps": np.ascontiguousarray(beps[:, rs]),
            }
        )

    res = run_bass_kernel_spmd(nc, in_maps, core_ids=list(range(N_CORES)))
    return np.concatenate([res.results[c]["y"] for c in range(N_CORES)], axis=1)


# revision 7
# speedup vs baseline: 1.3783x; 1.3783x over previous
"""BayesianLinear (y = x @ (mu + softplus(rho) * eps).T + bias) on 8 TRN2 cores.

Column-parallel sharding: each core owns OUT_F/8 = 512 output features.

Host-side prep is pure layout/precision staging (no reference math):
  - x is cast to bf16 and pre-tiled into the SBUF layout the TensorEngine
    needs for its stationary operand: x_t[bt, pi, po, bi] = x[bt*128+bi,
    po*128+pi], so each 128-row batch tile is one contiguous 1 MiB DMA.
  - weight_mu/rho/eps shards are transposed to [in_f, o_sh] and tiled as
    [in_f/128, 128, o_sh] so W^T can be constructed on-device directly in
    its matmul layout (K on partitions), one contiguous 256 KiB DMA per
    K-block per param.

Device per core:
  1. bias row = bias_mu + softplus(bias_rho) * bias_eps (fp32 ACT+DVE,
     cast bf16). softplus(v) = Ln(1 + Exp(v)) — no Softplus LUT on TRN2;
     Exp and Ln share one ACT table.
  2. For each K-block k (32): load the three param tiles [128, 512] fp32,
     softplus+mul+add on ACT/DVE, write bf16 straight into the resident
     W^T tile [128, 32, 512]. No DRAM bounce, no transpose on device.
  3. For each batch tile bt (64): one linear DMA for x^T [128, 32, 128]
     bf16, 32 accumulating bf16 matmuls into PSUM [128, 512] fp32, bias
     added via a K=1 matmul against a ones row, DVE eviction, DMA out.
  Matmuls for K-block k only depend on W^T block k, so the weight
  construction overlaps the start of the matmul stream.
"""

import numpy as np
import ml_dtypes

import concourse.bacc as bacc
import concourse.mybir as mybir
import concourse.tile as tile
from concourse.bass_utils import run_bass_kernel_spmd

BATCH = 8192
IN_F = 4096
OUT_F = 4096
N_CORES = 8
P = 128

_NC_CACHE = {}


def build_nc(batch=BATCH, in_f=IN_F, o_sh=OUT_F // N_CORES):
    KB = in_f // P  # K-blocks of 128 along the contraction dim
    BT = batch // P  # 128-row output tiles

    nc = bacc.Bacc(
        "TRN2",
        target_bir_lowering=False,
        debug=False,
        enable_asserts=False,
        num_devices=N_CORES,
    )
    bf16 = mybir.dt.bfloat16
    f32 = mybir.dt.float32

    x = nc.declare_dram_parameter("x_t", [BT, P, KB, P], bf16, isOutput=False)
    wmu = nc.declare_dram_parameter("wmu_t", [KB, P, o_sh], f32, isOutput=False)
    wrho = nc.declare_dram_parameter("wrho_t", [KB, P, o_sh], f32, isOutput=False)
    weps = nc.declare_dram_parameter("weps_t", [KB, P, o_sh], f32, isOutput=False)
    bmu = nc.declare_dram_parameter("bias_mu", [1, o_sh], f32, isOutput=False)
    brho = nc.declare_dram_parameter("bias_rho", [1, o_sh], f32, isOutput=False)
    beps = nc.declare_dram_parameter("bias_eps", [1, o_sh], f32, isOutput=False)
    y = nc.declare_dram_parameter("y", [batch, o_sh], f32, isOutput=True)

    act_exp = mybir.ActivationFunctionType.Exp
    act_ln = mybir.ActivationFunctionType.Ln

    with tile.TileContext(nc) as tc:
        with (
            tc.tile_pool(name="const", bufs=1) as const,
            tc.tile_pool(name="wcons", bufs=3) as wcons,
            tc.tile_pool(name="xin", bufs=4) as xin,
            tc.tile_pool(name="yout", bufs=4) as yout,
            tc.tile_pool(name="psum", bufs=4, space="PSUM") as psum_pool,
        ):
            # ---- bias row and ones row for the K=1 bias matmul
            b_mu = const.tile([1, o_sh], f32, tag="b_mu")
            b_rho = const.tile([1, o_sh], f32, tag="b_rho")
            b_eps = const.tile([1, o_sh], f32, tag="b_eps")
            nc.sync.dma_start(out=b_mu[:], in_=bmu[:])
            nc.sync.dma_start(out=b_rho[:], in_=brho[:])
            nc.sync.dma_start(out=b_eps[:], in_=beps[:])
            b_sp = const.tile([1, o_sh], f32, tag="b_sp")
            nc.scalar.activation(b_sp[:], b_rho[:], act_exp)
            nc.scalar.activation(b_sp[:], b_sp[:], act_ln, bias=1.0)
            nc.vector.tensor_mul(out=b_sp[:], in0=b_sp[:], in1=b_eps[:])
            bias_bf = const.tile([1, o_sh], bf16, tag="bias_bf")
            nc.vector.tensor_add(out=bias_bf[:], in0=b_sp[:], in1=b_mu[:])
            ones = const.tile([1, P], bf16, tag="ones")
            nc.vector.memset(ones[:], 1.0)

            # ---- W^T constructed in place, one K-block at a time
            WT = const.tile([P, KB, o_sh], bf16, tag="WT")
            for k in range(KB):
                mu_t = wcons.tile([P, o_sh], f32, tag="mu")
                rho_t = wcons.tile([P, o_sh], f32, tag="rho")
                eps_t = wcons.tile([P, o_sh], f32, tag="eps")
                nc.sync.dma_start(out=mu_t[:], in_=wmu[k])
                nc.sync.dma_start(out=rho_t[:], in_=wrho[k])
                nc.sync.dma_start(out=eps_t[:], in_=weps[k])
                sp_t = wcons.tile([P, o_sh], f32, tag="sp")
                nc.scalar.activation(sp_t[:], rho_t[:], act_exp)
                nc.scalar.activation(sp_t[:], sp_t[:], act_ln, bias=1.0)
                nc.vector.tensor_mul(out=sp_t[:], in0=sp_t[:], in1=eps_t[:])
                nc.vector.tensor_add(out=WT[:, k, :], in0=sp_t[:], in1=mu_t[:])

            # ---- main loop over 128-row output tiles
            for bt in range(BT):
                xT = xin.tile([P, KB, P], bf16, tag="xT")
                nc.sync.dma_start(out=xT[:], in_=x[bt])
                ps = psum_pool.tile([P, o_sh], f32, tag="ps")
                for k in range(KB):
                    nc.tensor.matmul(
                        ps[:],
                        lhsT=xT[:, k, :],
                        rhs=WT[:, k, :],
                        start=(k == 0),
                        stop=False,
                    )
                nc.tensor.matmul(
                    ps[:], lhsT=ones[:], rhs=bias_bf[:], start=False, stop=True
                )
                y_sb = yout.tile([P, o_sh], f32, tag="y_sb")
                nc.vector.tensor_copy(out=y_sb[:], in_=ps[:])
                nc.sync.dma_start(out=y[bt * P : (bt + 1) * P, :], in_=y_sb[:])

    # Skip bacc's pre-placed InstLoadActFuncSet: on large graphs walrus's
    # parallel-pass fork can separate the hoisted load from its activations
    # ("No Act func set exist for this instruction"); walrus's own lower_act
    # placement handles forked subgraphs correctly.
    nc.insert_act_table_loads = lambda: None
    nc.compile()
    return nc


def _prep_x(x):
    """[batch, in_f] fp32 -> bf16 tiled [BT, 128, KB, 128] with
    x_t[bt, pi, po, bi] = x[bt*128 + bi, po*128 + pi]."""
    batch, in_f = x.shape
    xb = x.astype(ml_dtypes.bfloat16)
    xb = xb.reshape(batch // P, P, in_f // P, P)  # [bt, bi, po, pi]
    return np.ascontiguousarray(xb.transpose(0, 3, 2, 1))  # [bt, pi, po, bi]


def _prep_w(w):
    """[o_sh, in_f] fp32 -> fp32 tiled [KB, 128, o_sh] with
    w_t[k, pi, o] = w[o, k*128 + pi]."""
    o_sh, in_f = w.shape
    wt = w.T.reshape(in_f // P, P, o_sh)  # [k, pi, o]
    return np.ascontiguousarray(wt)


def make_in_maps(x, weight_mu, weight_rho, bias_mu, bias_rho, weight_eps, bias_eps):
    o_sh = OUT_F // N_CORES
    x_t = _prep_x(np.asarray(x, dtype=np.float32))
    wmu = np.asarray(weight_mu, dtype=np.float32)
    wrho = np.asarray(weight_rho, dtype=np.float32)
    weps = np.asarray(weight_eps, dtype=np.float32)
    bmu = np.asarray(bias_mu, dtype=np.float32).reshape(1, -1)
    brho = np.asarray(bias_rho, dtype=np.float32).reshape(1, -1)
    beps = np.asarray(bias_eps, dtype=np.float32).reshape(1, -1)

    in_maps = []
    for c in range(N_CORES):
        rs = slice(c * o_sh, (c + 1) * o_sh)
        in_maps.append(
            {
                "x_t": x_t,
                "wmu_t": _prep_w(wmu[rs]),
                "wrho_t": _prep_w(wrho[rs]),
                "weps_t": _prep_w(weps[rs]),
                "bias_mu": np.ascontiguousarray(bmu[:, rs]),
                "bias_rho": np.ascontiguousarray(brho[:, rs]),
                "bias_eps": np.ascontiguousarray(beps[:, rs]),
            }
        )
    return in_maps


def kernel(x, weight_mu, weight_rho, bias_mu, bias_rho, weight_eps, bias_eps):
    o_sh = OUT_F // N_CORES
    key = (x.shape, o_sh)
    if key not in _NC_CACHE:
        _NC_CACHE[key] = build_nc(x.shape[0], x.shape[1], o_sh)
    nc = _NC_CACHE[key]

    in_maps = make_in_maps(
        x, weight_mu, weight_rho, bias_mu, bias_rho, weight_eps, bias_eps
    )
    res = run_bass_kernel_spmd(nc, in_maps, core_ids=list(range(N_CORES)))
    return np.concatenate([res.results[c]["y"] for c in range(N_CORES)], axis=1)


# revision 13
# speedup vs baseline: 1.4531x; 1.0543x over previous
"""BayesianLinear (y = x @ (mu + softplus(rho) * eps).T + bias) on 8 TRN2 cores.

Column-parallel sharding: each core owns OUT_F/8 = 512 output features.

Host-side prep is pure layout/precision staging (no reference math):
  - x is cast to bf16 and pre-tiled into the SBUF layout the TensorEngine
    needs for its stationary operand: x_t[bt, pi, po, bi] = x[bt*128+bi,
    po*128+pi], so each 128-row batch tile is one contiguous 1 MiB DMA.
  - weight_mu/rho/eps shards are transposed to [in_f, o_sh] and tiled as
    [in_f/128, 128, o_sh] so W^T can be constructed on-device directly in
    its matmul layout (K on partitions), one contiguous 256 KiB DMA per
    K-block per param.

Device per core:
  1. bias row = bias_mu + softplus(bias_rho) * bias_eps (fp32 ACT+DVE,
     cast bf16). softplus(v) = Ln(1 + Exp(v)) — no Softplus LUT on TRN2;
     Exp and Ln share one ACT table.
  2. For each K-block k (32): load the three param tiles [128, 512] fp32,
     softplus+mul+add on ACT/DVE, write bf16 straight into the resident
     W^T tile [128, 32, 512]. No DRAM bounce, no transpose on device.
  3. For each batch tile bt (64): one linear DMA for x^T [128, 32, 128]
     bf16, 32 accumulating bf16 matmuls into PSUM [128, 512] fp32, bias
     added via a K=1 matmul against a ones row, DVE eviction, DMA out.
  Matmuls for K-block k only depend on W^T block k, so the weight
  construction overlaps the start of the matmul stream.
"""

import numpy as np
import ml_dtypes

import concourse.bacc as bacc
import concourse.mybir as mybir
import concourse.tile as tile
from concourse.bass_utils import run_bass_kernel_spmd

BATCH = 8192
IN_F = 4096
OUT_F = 4096
N_CORES = 8
P = 128

_NC_CACHE = {}


def build_nc(batch=BATCH, in_f=IN_F, o_sh=OUT_F // N_CORES):
    KB = in_f // P  # K-blocks of 128 along the contraction dim
    BT = batch // P  # 128-row output tiles

    nc = bacc.Bacc(
        "TRN2",
        target_bir_lowering=False,
        debug=False,
        enable_asserts=False,
        num_devices=N_CORES,
    )
    bf16 = mybir.dt.bfloat16
    f32 = mybir.dt.float32

    x = nc.declare_dram_parameter("x_t", [BT, P, KB, P], bf16, isOutput=False)
    # mu/eps ship as bf16 (their info is rounded into the bf16 W anyway);
    # rho stays fp32 — softplus amplifies its quantization error ~3x.
    wmu = nc.declare_dram_parameter("wmu_t", [KB, P, o_sh], bf16, isOutput=False)
    wrho = nc.declare_dram_parameter("wrho_t", [KB, P, o_sh], f32, isOutput=False)
    weps = nc.declare_dram_parameter("weps_t", [KB, P, o_sh], bf16, isOutput=False)
    bmu = nc.declare_dram_parameter("bias_mu", [1, o_sh], f32, isOutput=False)
    brho = nc.declare_dram_parameter("bias_rho", [1, o_sh], f32, isOutput=False)
    beps = nc.declare_dram_parameter("bias_eps", [1, o_sh], f32, isOutput=False)
    y = nc.declare_dram_parameter("y", [batch, o_sh], f32, isOutput=True)

    act_exp = mybir.ActivationFunctionType.Exp
    act_ln = mybir.ActivationFunctionType.Ln

    with tile.TileContext(nc) as tc:
        with (
            tc.tile_pool(name="const", bufs=1) as const,
            tc.tile_pool(name="wcons", bufs=3) as wcons,
            tc.tile_pool(name="xin", bufs=10) as xin,
            tc.tile_pool(name="yout", bufs=4) as yout,
            tc.tile_pool(name="psum", bufs=8, space="PSUM") as psum_pool,
        ):
            # ---- bias row and ones row for the K=1 bias matmul
            b_mu = const.tile([1, o_sh], f32, tag="b_mu")
            b_rho = const.tile([1, o_sh], f32, tag="b_rho")
            b_eps = const.tile([1, o_sh], f32, tag="b_eps")
            nc.sync.dma_start(out=b_mu[:], in_=bmu[:])
            nc.sync.dma_start(out=b_rho[:], in_=brho[:])
            nc.sync.dma_start(out=b_eps[:], in_=beps[:])
            b_sp = const.tile([1, o_sh], f32, tag="b_sp")
            nc.scalar.activation(b_sp[:], b_rho[:], act_exp)
            nc.scalar.activation(b_sp[:], b_sp[:], act_ln, bias=1.0)
            nc.vector.tensor_mul(out=b_sp[:], in0=b_sp[:], in1=b_eps[:])
            bias_bf = const.tile([1, o_sh], bf16, tag="bias_bf")
            nc.vector.tensor_add(out=bias_bf[:], in0=b_sp[:], in1=b_mu[:])
            ones = const.tile([1, P], bf16, tag="ones")
            nc.vector.memset(ones[:], 1.0)

            # ---- W^T constructed in place, one K-block at a time.
            # Param DMAs ride the ACT HWDGE queue so they don't head-of-line
            # block the x-tile loads on the sync queue.
            WT = const.tile([P, KB, o_sh], bf16, tag="WT")
            for k in range(KB):
                mu_t = wcons.tile([P, o_sh], bf16, tag="mu")
                rho_t = wcons.tile([P, o_sh], f32, tag="rho")
                eps_t = wcons.tile([P, o_sh], bf16, tag="eps")
                nc.scalar.dma_start(out=mu_t[:], in_=wmu[k])
                nc.scalar.dma_start(out=rho_t[:], in_=wrho[k])
                nc.scalar.dma_start(out=eps_t[:], in_=weps[k])
                sp_t = wcons.tile([P, o_sh], f32, tag="sp")
                nc.scalar.activation(sp_t[:], rho_t[:], act_exp)
                nc.scalar.activation(sp_t[:], sp_t[:], act_ln, bias=1.0)
                nc.vector.tensor_mul(out=sp_t[:], in0=sp_t[:], in1=eps_t[:])
                nc.vector.tensor_add(out=WT[:, k, :], in0=sp_t[:], in1=mu_t[:])

            def body_tail(ps, bt):
                nc.tensor.matmul(
                    ps[:], lhsT=ones[:], rhs=bias_bf[:], start=False, stop=True
                )
                y_sb = yout.tile([P, o_sh], f32, tag="y_sb")
                nc.vector.tensor_copy(out=y_sb[:], in_=ps[:])
                nc.sync.dma_start(out=y[bt * P : (bt + 1) * P, :], in_=y_sb[:])

            # ---- first GROUP tiles run k-interleaved across PSUM banks so
            # the PE consumes W^T blocks no faster than construction makes
            # them — the weight-construction latency hides under matmuls.
            GROUP = min(8, BT)
            xts = []
            pss = []
            for bt in range(GROUP):
                xT = xin.tile([P, KB, P], bf16, tag="xT")
                nc.sync.dma_start(out=xT[:], in_=x[bt])
                xts.append(xT)
                ps = psum_pool.tile([P, o_sh], f32, tag="ps", name=f"ps_g{bt}")
                pss.append(ps)
            for k in range(KB):
                for i in range(GROUP):
                    nc.tensor.matmul(
                        pss[i][:],
                        lhsT=xts[i][:, k, :],
                        rhs=WT[:, k, :],
                        start=(k == 0),
                        stop=False,
                    )
            for i in range(GROUP):
                body_tail(pss[i], i)

            # ---- remaining tiles stream one PSUM bank each
            for bt in range(GROUP, BT):
                xT = xin.tile([P, KB, P], bf16, tag="xT")
                nc.sync.dma_start(out=xT[:], in_=x[bt])
                ps = psum_pool.tile([P, o_sh], f32, tag="ps")
                for k in range(KB):
                    nc.tensor.matmul(
                        ps[:],
                        lhsT=xT[:, k, :],
                        rhs=WT[:, k, :],
                        start=(k == 0),
                        stop=False,
                    )
                body_tail(ps, bt)

    # Skip bacc's pre-placed InstLoadActFuncSet: on large graphs walrus's
    # parallel-pass fork can separate the hoisted load from its activations
    # ("No Act func set exist for this instruction"); walrus's own lower_act
    # placement handles forked subgraphs correctly.
    nc.insert_act_table_loads = lambda: None
    nc.compile()
    return nc


def _prep_x(x):
    """[batch, in_f] fp32 -> bf16 tiled [BT, 128, KB, 128] with
    x_t[bt, pi, po, bi] = x[bt*128 + bi, po*128 + pi]."""
    batch, in_f = x.shape
    xb = x.astype(ml_dtypes.bfloat16)
    xb = xb.reshape(batch // P, P, in_f // P, P)  # [bt, bi, po, pi]
    return np.ascontiguousarray(xb.transpose(0, 3, 2, 1))  # [bt, pi, po, bi]


def _prep_w(w, dtype=np.float32):
    """[o_sh, in_f] -> tiled [KB, 128, o_sh] with w_t[k, pi, o] = w[o, k*128 + pi]."""
    o_sh, in_f = w.shape
    wt = w.T.reshape(in_f // P, P, o_sh)  # [k, pi, o]
    return np.ascontiguousarray(wt).astype(dtype)


def make_in_maps(x, weight_mu, weight_rho, bias_mu, bias_rho, weight_eps, bias_eps):
    o_sh = OUT_F // N_CORES
    x_t = _prep_x(np.asarray(x, dtype=np.float32))
    wmu = np.asarray(weight_mu, dtype=np.float32)
    wrho = np.asarray(weight_rho, dtype=np.float32)
    weps = np.asarray(weight_eps, dtype=np.float32)
    bmu = np.asarray(bias_mu, dtype=np.float32).reshape(1, -1)
    brho = np.asarray(bias_rho, dtype=np.float32).reshape(1, -1)
    beps = np.asarray(bias_eps, dtype=np.float32).reshape(1, -1)

    in_maps = []
    for c in range(N_CORES):
        rs = slice(c * o_sh, (c + 1) * o_sh)
        in_maps.append(
            {
                "x_t": x_t,
                "wmu_t": _prep_w(wmu[rs], ml_dtypes.bfloat16),
                "wrho_t": _prep_w(wrho[rs]),
                "weps_t": _prep_w(weps[rs], ml_dtypes.bfloat16),
                "bias_mu": np.ascontiguousarray(bmu[:, rs]),
                "bias_rho": np.ascontiguousarray(brho[:, rs]),
                "bias_eps": np.ascontiguousarray(beps[:, rs]),
            }
        )
    return in_maps


def kernel(x, weight_mu, weight_rho, bias_mu, bias_rho, weight_eps, bias_eps):
    o_sh = OUT_F // N_CORES
    key = (x.shape, o_sh)
    if key not in _NC_CACHE:
        _NC_CACHE[key] = build_nc(x.shape[0], x.shape[1], o_sh)
    nc = _NC_CACHE[key]

    in_maps = make_in_maps(
        x, weight_mu, weight_rho, bias_mu, bias_rho, weight_eps, bias_eps
    )
    res = run_bass_kernel_spmd(nc, in_maps, core_ids=list(range(N_CORES)))
    return np.concatenate([res.results[c]["y"] for c in range(N_CORES)], axis=1)


# revision 14
# speedup vs baseline: 1.5168x; 1.0438x over previous
"""BayesianLinear (y = x @ (mu + softplus(rho) * eps).T + bias) on 8 TRN2 cores.

Column-parallel sharding: each core owns OUT_F/8 = 512 output features.

Host-side prep is pure layout/precision staging (no reference math):
  - x is cast to bf16 and pre-tiled into the SBUF layout the TensorEngine
    needs for its stationary operand: x_t[bt, pi, po, bi] = x[bt*128+bi,
    po*128+pi], so each 128-row batch tile is one contiguous 1 MiB DMA.
  - weight_mu/rho/eps shards are transposed to [in_f, o_sh] and tiled as
    [in_f/128, 128, o_sh] so W^T can be constructed on-device directly in
    its matmul layout (K on partitions), one contiguous 256 KiB DMA per
    K-block per param.

Device per core:
  1. bias row = bias_mu + softplus(bias_rho) * bias_eps (fp32 ACT+DVE,
     cast bf16). softplus(v) = Ln(1 + Exp(v)) — no Softplus LUT on TRN2;
     Exp and Ln share one ACT table.
  2. For each K-block k (32): load the three param tiles [128, 512] fp32,
     softplus+mul+add on ACT/DVE, write bf16 straight into the resident
     W^T tile [128, 32, 512]. No DRAM bounce, no transpose on device.
  3. For each batch tile bt (64): one linear DMA for x^T [128, 32, 128]
     bf16, 32 accumulating bf16 matmuls into PSUM [128, 512] fp32, bias
     added via a K=1 matmul against a ones row, DVE eviction, DMA out.
  Matmuls for K-block k only depend on W^T block k, so the weight
  construction overlaps the start of the matmul stream.
"""

import numpy as np
import ml_dtypes

import concourse.bacc as bacc
import concourse.mybir as mybir
import concourse.tile as tile
from concourse.bass_utils import run_bass_kernel_spmd

BATCH = 8192
IN_F = 4096
OUT_F = 4096
N_CORES = 8
P = 128

_NC_CACHE = {}


def build_nc(batch=BATCH, in_f=IN_F, o_sh=OUT_F // N_CORES):
    KB = in_f // P  # K-blocks of 128 along the contraction dim
    BT = batch // P  # 128-row output tiles

    nc = bacc.Bacc(
        "TRN2",
        target_bir_lowering=False,
        debug=False,
        enable_asserts=False,
        num_devices=N_CORES,
    )
    bf16 = mybir.dt.bfloat16
    f32 = mybir.dt.float32

    x = nc.declare_dram_parameter("x_t", [BT, P, KB, P], bf16, isOutput=False)
    # mu/eps ship as bf16 (their info is rounded into the bf16 W anyway);
    # rho ships as fp16 — softplus amplifies quantization ~3x, and fp16's
    # 10-bit mantissa keeps that error negligible at half the fp32 bytes.
    f16 = mybir.dt.float16
    wmu = nc.declare_dram_parameter("wmu_t", [KB, P, o_sh], bf16, isOutput=False)
    wrho = nc.declare_dram_parameter("wrho_t", [KB, P, o_sh], f16, isOutput=False)
    weps = nc.declare_dram_parameter("weps_t", [KB, P, o_sh], bf16, isOutput=False)
    bmu = nc.declare_dram_parameter("bias_mu", [1, o_sh], f32, isOutput=False)
    brho = nc.declare_dram_parameter("bias_rho", [1, o_sh], f32, isOutput=False)
    beps = nc.declare_dram_parameter("bias_eps", [1, o_sh], f32, isOutput=False)
    y = nc.declare_dram_parameter("y", [batch, o_sh], f32, isOutput=True)

    act_exp = mybir.ActivationFunctionType.Exp
    act_ln = mybir.ActivationFunctionType.Ln

    with tile.TileContext(nc) as tc:
        with (
            tc.tile_pool(name="const", bufs=1) as const,
            tc.tile_pool(name="wcons", bufs=3) as wcons,
            tc.tile_pool(name="xin", bufs=10) as xin,
            tc.tile_pool(name="yout", bufs=4) as yout,
            tc.tile_pool(name="psum", bufs=8, space="PSUM") as psum_pool,
        ):
            # ---- bias row and ones row for the K=1 bias matmul
            b_mu = const.tile([1, o_sh], f32, tag="b_mu")
            b_rho = const.tile([1, o_sh], f32, tag="b_rho")
            b_eps = const.tile([1, o_sh], f32, tag="b_eps")
            nc.sync.dma_start(out=b_mu[:], in_=bmu[:])
            nc.sync.dma_start(out=b_rho[:], in_=brho[:])
            nc.sync.dma_start(out=b_eps[:], in_=beps[:])
            b_sp = const.tile([1, o_sh], f32, tag="b_sp")
            nc.scalar.activation(b_sp[:], b_rho[:], act_exp)
            nc.scalar.activation(b_sp[:], b_sp[:], act_ln, bias=1.0)
            nc.vector.tensor_mul(out=b_sp[:], in0=b_sp[:], in1=b_eps[:])
            bias_bf = const.tile([1, o_sh], bf16, tag="bias_bf")
            nc.vector.tensor_add(out=bias_bf[:], in0=b_sp[:], in1=b_mu[:])
            ones = const.tile([1, P], bf16, tag="ones")
            nc.vector.memset(ones[:], 1.0)

            # ---- W^T constructed in place, one K-block at a time.
            # Param DMAs ride the ACT HWDGE queue so they don't head-of-line
            # block the x-tile loads on the sync queue.
            WT = const.tile([P, KB, o_sh], bf16, tag="WT")
            for k in range(KB):
                mu_t = wcons.tile([P, o_sh], bf16, tag="mu")
                rho_t = wcons.tile([P, o_sh], f16, tag="rho")
                eps_t = wcons.tile([P, o_sh], bf16, tag="eps")
                nc.scalar.dma_start(out=mu_t[:], in_=wmu[k])
                nc.scalar.dma_start(out=rho_t[:], in_=wrho[k])
                nc.scalar.dma_start(out=eps_t[:], in_=weps[k])
                sp_t = wcons.tile([P, o_sh], f32, tag="sp")
                nc.scalar.activation(sp_t[:], rho_t[:], act_exp)
                nc.scalar.activation(sp_t[:], sp_t[:], act_ln, bias=1.0)
                nc.vector.tensor_mul(out=sp_t[:], in0=sp_t[:], in1=eps_t[:])
                nc.vector.tensor_add(out=WT[:, k, :], in0=sp_t[:], in1=mu_t[:])

            def body_tail(ps, bt):
                nc.tensor.matmul(
                    ps[:], lhsT=ones[:], rhs=bias_bf[:], start=False, stop=True
                )
                y_sb = yout.tile([P, o_sh], f32, tag="y_sb")
                nc.vector.tensor_copy(out=y_sb[:], in_=ps[:])
                nc.sync.dma_start(out=y[bt * P : (bt + 1) * P, :], in_=y_sb[:])

            # ---- first GROUP tiles run k-interleaved across PSUM banks so
            # the PE consumes W^T blocks no faster than construction makes
            # them — the weight-construction latency hides under matmuls.
            GROUP = min(8, BT)
            xts = []
            pss = []
            for bt in range(GROUP):
                xT = xin.tile([P, KB, P], bf16, tag="xT")
                nc.sync.dma_start(out=xT[:], in_=x[bt])
                xts.append(xT)
                ps = psum_pool.tile([P, o_sh], f32, tag="ps", name=f"ps_g{bt}")
                pss.append(ps)
            for k in range(KB):
                for i in range(GROUP):
                    nc.tensor.matmul(
                        pss[i][:],
                        lhsT=xts[i][:, k, :],
                        rhs=WT[:, k, :],
                        start=(k == 0),
                        stop=False,
                    )
            for i in range(GROUP):
                body_tail(pss[i], i)

            # ---- remaining tiles stream one PSUM bank each
            for bt in range(GROUP, BT):
                xT = xin.tile([P, KB, P], bf16, tag="xT")
                nc.sync.dma_start(out=xT[:], in_=x[bt])
                ps = psum_pool.tile([P, o_sh], f32, tag="ps")
                for k in range(KB):
                    nc.tensor.matmul(
                        ps[:],
                        lhsT=xT[:, k, :],
                        rhs=WT[:, k, :],
                        start=(k == 0),
                        stop=False,
                    )
                body_tail(ps, bt)

    # Skip bacc's pre-placed InstLoadActFuncSet: on large graphs walrus's
    # parallel-pass fork can separate the hoisted load from its activations
    # ("No Act func set exist for this instruction"); walrus's own lower_act
    # placement handles forked subgraphs correctly.
    nc.insert_act_table_loads = lambda: None
    nc.compile()
    return nc


def _prep_x(x):
    """[batch, in_f] fp32 -> bf16 tiled [BT, 128, KB, 128] with
    x_t[bt, pi, po, bi] = x[bt*128 + bi, po*128 + pi]."""
    batch, in_f = x.shape
    xb = x.astype(ml_dtypes.bfloat16)
    xb = xb.reshape(batch // P, P, in_f // P, P)  # [bt, bi, po, pi]
    return np.ascontiguousarray(xb.transpose(0, 3, 2, 1))  # [bt, pi, po, bi]


def _prep_w(w, dtype=np.float32):
    """[o_sh, in_f] -> tiled [KB, 128, o_sh] with w_t[k, pi, o] = w[o, k*128 + pi]."""
    o_sh, in_f = w.shape
    wt = w.T.reshape(in_f // P, P, o_sh)  # [k, pi, o]
    return np.ascontiguousarray(wt).astype(dtype)


def make_in_maps(x, weight_mu, weight_rho, bias_mu, bias_rho, weight_eps, bias_eps):
    o_sh = OUT_F // N_CORES
    x_t = _prep_x(np.asarray(x, dtype=np.float32))
    wmu = np.asarray(weight_mu, dtype=np.float32)
    wrho = np.asarray(weight_rho, dtype=np.float32)
    weps = np.asarray(weight_eps, dtype=np.float32)
    bmu = np.asarray(bias_mu, dtype=np.float32).reshape(1, -1)
    brho = np.asarray(bias_rho, dtype=np.float32).reshape(1, -1)
    beps = np.asarray(bias_eps, dtype=np.float32).reshape(1, -1)

    in_maps = []
    for c in range(N_CORES):
        rs = slice(c * o_sh, (c + 1) * o_sh)
        in_maps.append(
            {
                "x_t": x_t,
                "wmu_t": _prep_w(wmu[rs], ml_dtypes.bfloat16),
                "wrho_t": _prep_w(wrho[rs], np.float16),
                "weps_t": _prep_w(weps[rs], ml_dtypes.bfloat16),
                "bias_mu": np.ascontiguousarray(bmu[:, rs]),
                "bias_rho": np.ascontiguousarray(brho[:, rs]),
                "bias_eps": np.ascontiguousarray(beps[:, rs]),
            }
        )
    return in_maps


def kernel(x, weight_mu, weight_rho, bias_mu, bias_rho, weight_eps, bias_eps):
    o_sh = OUT_F // N_CORES
    key = (x.shape, o_sh)
    if key not in _NC_CACHE:
        _NC_CACHE[key] = build_nc(x.shape[0], x.shape[1], o_sh)
    nc = _NC_CACHE[key]

    in_maps = make_in_maps(
        x, weight_mu, weight_rho, bias_mu, bias_rho, weight_eps, bias_eps
    )
    res = run_bass_kernel_spmd(nc, in_maps, core_ids=list(range(N_CORES)))
    return np.concatenate([res.results[c]["y"] for c in range(N_CORES)], axis=1)


# revision 15
# speedup vs baseline: 1.6305x; 1.0750x over previous
"""BayesianLinear (y = x @ (mu + softplus(rho) * eps).T + bias) on 8 TRN2 cores.

Column-parallel sharding: each core owns OUT_F/8 = 512 output features.

Host-side prep is pure layout/precision staging (no reference math):
  - x is cast to bf16 and pre-tiled into the SBUF layout the TensorEngine
    needs for its stationary operand: x_t[bt, pi, po, bi] = x[bt*128+bi,
    po*128+pi], so each 128-row batch tile is one contiguous 1 MiB DMA.
  - weight_mu/rho/eps shards are transposed to [in_f, o_sh] and tiled as
    [in_f/128, 128, o_sh] so W^T can be constructed on-device directly in
    its matmul layout (K on partitions), one contiguous 256 KiB DMA per
    K-block per param.

Device per core:
  1. bias row = bias_mu + softplus(bias_rho) * bias_eps (fp32 ACT+DVE,
     cast bf16). softplus(v) = Ln(1 + Exp(v)) — no Softplus LUT on TRN2;
     Exp and Ln share one ACT table.
  2. For each K-block k (32): load the three param tiles [128, 512] fp32,
     softplus+mul+add on ACT/DVE, write bf16 straight into the resident
     W^T tile [128, 32, 512]. No DRAM bounce, no transpose on device.
  3. For each batch tile bt (64): one linear DMA for x^T [128, 32, 128]
     bf16, 32 accumulating bf16 matmuls into PSUM [128, 512] fp32, bias
     added via a K=1 matmul against a ones row, DVE eviction, DMA out.
  Matmuls for K-block k only depend on W^T block k, so the weight
  construction overlaps the start of the matmul stream.
"""

import numpy as np
import ml_dtypes

import concourse.bacc as bacc
import concourse.mybir as mybir
import concourse.tile as tile
from concourse.bass_utils import run_bass_kernel_spmd

BATCH = 8192
IN_F = 4096
OUT_F = 4096
N_CORES = 8
P = 128

_NC_CACHE = {}


def build_nc(batch=BATCH, in_f=IN_F, o_sh=OUT_F // N_CORES):
    KB = in_f // P  # K-blocks of 128 along the contraction dim
    BT = batch // P  # 128-row output tiles

    nc = bacc.Bacc(
        "TRN2",
        target_bir_lowering=False,
        debug=False,
        enable_asserts=False,
        num_devices=N_CORES,
    )
    bf16 = mybir.dt.bfloat16
    f32 = mybir.dt.float32

    x = nc.declare_dram_parameter("x_t", [BT, P, KB, P], bf16, isOutput=False)
    # mu/eps ship as bf16 (their info is rounded into the bf16 W anyway);
    # rho ships as fp16 — softplus amplifies quantization ~3x, and fp16's
    # 10-bit mantissa keeps that error negligible at half the fp32 bytes.
    f16 = mybir.dt.float16
    wmu = nc.declare_dram_parameter("wmu_t", [KB, P, o_sh], bf16, isOutput=False)
    wrho = nc.declare_dram_parameter("wrho_t", [KB, P, o_sh], f16, isOutput=False)
    weps = nc.declare_dram_parameter("weps_t", [KB, P, o_sh], bf16, isOutput=False)
    bmu = nc.declare_dram_parameter("bias_mu", [1, o_sh], f32, isOutput=False)
    brho = nc.declare_dram_parameter("bias_rho", [1, o_sh], f32, isOutput=False)
    beps = nc.declare_dram_parameter("bias_eps", [1, o_sh], f32, isOutput=False)
    y = nc.declare_dram_parameter("y", [batch, o_sh], f32, isOutput=True)

    act_exp = mybir.ActivationFunctionType.Exp
    act_ln = mybir.ActivationFunctionType.Ln

    with tile.TileContext(nc) as tc:
        with (
            tc.tile_pool(name="const", bufs=1) as const,
            tc.tile_pool(name="wcons", bufs=5) as wcons,
            tc.tile_pool(name="xin", bufs=10) as xin,
            tc.tile_pool(name="yout", bufs=4) as yout,
            tc.tile_pool(name="psum", bufs=8, space="PSUM") as psum_pool,
        ):
            # ---- bias row and ones row for the K=1 bias matmul
            b_mu = const.tile([1, o_sh], f32, tag="b_mu")
            b_rho = const.tile([1, o_sh], f32, tag="b_rho")
            b_eps = const.tile([1, o_sh], f32, tag="b_eps")
            nc.gpsimd.dma_start(out=b_mu[:], in_=bmu[:])
            nc.gpsimd.dma_start(out=b_rho[:], in_=brho[:])
            nc.gpsimd.dma_start(out=b_eps[:], in_=beps[:])
            b_sp = const.tile([1, o_sh], f32, tag="b_sp")
            nc.scalar.activation(b_sp[:], b_rho[:], act_exp)
            nc.scalar.activation(b_sp[:], b_sp[:], act_ln, bias=1.0)
            nc.vector.tensor_mul(out=b_sp[:], in0=b_sp[:], in1=b_eps[:])
            bias_bf = const.tile([1, o_sh], bf16, tag="bias_bf")
            nc.vector.tensor_add(out=bias_bf[:], in0=b_sp[:], in1=b_mu[:])
            ones = const.tile([1, P], bf16, tag="ones")
            nc.vector.memset(ones[:], 1.0)
            # broadcast bias across partitions once: [128, o_sh] = ones.T @ bias
            bias_ps = psum_pool.tile([P, o_sh], f32, tag="ps", name="bias_ps")
            nc.tensor.matmul(bias_ps[:], lhsT=ones[:], rhs=bias_bf[:])
            bias_sb = const.tile([P, o_sh], f32, tag="bias_sb")
            nc.vector.tensor_copy(out=bias_sb[:], in_=bias_ps[:])

            # ---- W^T constructed in place, one K-block at a time.
            # Param DMAs ride the GPSIMD SWDGE queue: the sync queue stays
            # dedicated to x-tile loads and the ACT engine to exp/ln.
            WT = const.tile([P, KB, o_sh], bf16, tag="WT")
            for k in range(KB):
                mu_t = wcons.tile([P, o_sh], bf16, tag="mu")
                rho_t = wcons.tile([P, o_sh], f16, tag="rho")
                eps_t = wcons.tile([P, o_sh], bf16, tag="eps")
                nc.gpsimd.dma_start(out=mu_t[:], in_=wmu[k])
                nc.gpsimd.dma_start(out=rho_t[:], in_=wrho[k])
                nc.gpsimd.dma_start(out=eps_t[:], in_=weps[k])
                sp_t = wcons.tile([P, o_sh], f32, tag="sp")
                nc.scalar.activation(sp_t[:], rho_t[:], act_exp)
                nc.scalar.activation(sp_t[:], sp_t[:], act_ln, bias=1.0)
                nc.vector.tensor_mul(out=sp_t[:], in0=sp_t[:], in1=eps_t[:])
                nc.vector.tensor_add(out=WT[:, k, :], in0=sp_t[:], in1=mu_t[:])

            def body_tail(ps, bt):
                y_sb = yout.tile([P, o_sh], f32, tag="y_sb")
                nc.vector.tensor_add(out=y_sb[:], in0=ps[:], in1=bias_sb[:])
                nc.sync.dma_start(out=y[bt * P : (bt + 1) * P, :], in_=y_sb[:])

            # ---- first GROUP tiles run k-interleaved across PSUM banks so
            # the PE consumes W^T blocks no faster than construction makes
            # them — the weight-construction latency hides under matmuls.
            GROUP = min(8, BT)
            xts = []
            pss = []
            for bt in range(GROUP):
                xT = xin.tile([P, KB, P], bf16, tag="xT")
                nc.sync.dma_start(out=xT[:], in_=x[bt])
                xts.append(xT)
                ps = psum_pool.tile([P, o_sh], f32, tag="ps", name=f"ps_g{bt}")
                pss.append(ps)
            for k in range(KB):
                for i in range(GROUP):
                    nc.tensor.matmul(
                        pss[i][:],
                        lhsT=xts[i][:, k, :],
                        rhs=WT[:, k, :],
                        start=(k == 0),
                        stop=(k == KB - 1),
                    )
            for i in range(GROUP):
                body_tail(pss[i], i)

            # ---- remaining tiles stream one PSUM bank each
            for bt in range(GROUP, BT):
                xT = xin.tile([P, KB, P], bf16, tag="xT")
                nc.sync.dma_start(out=xT[:], in_=x[bt])
                ps = psum_pool.tile([P, o_sh], f32, tag="ps")
                for k in range(KB):
                    nc.tensor.matmul(
                        ps[:],
                        lhsT=xT[:, k, :],
                        rhs=WT[:, k, :],
                        start=(k == 0),
                        stop=(k == KB - 1),
                    )
                body_tail(ps, bt)

    # Skip bacc's pre-placed InstLoadActFuncSet: on large graphs walrus's
    # parallel-pass fork can separate the hoisted load from its activations
    # ("No Act func set exist for this instruction"); walrus's own lower_act
    # placement handles forked subgraphs correctly.
    nc.insert_act_table_loads = lambda: None
    nc.compile()
    return nc


def _prep_x(x):
    """[batch, in_f] fp32 -> bf16 tiled [BT, 128, KB, 128] with
    x_t[bt, pi, po, bi] = x[bt*128 + bi, po*128 + pi]."""
    batch, in_f = x.shape
    xb = x.astype(ml_dtypes.bfloat16)
    xb = xb.reshape(batch // P, P, in_f // P, P)  # [bt, bi, po, pi]
    return np.ascontiguousarray(xb.transpose(0, 3, 2, 1))  # [bt, pi, po, bi]


def _prep_w(w, dtype=np.float32):
    """[o_sh, in_f] -> tiled [KB, 128, o_sh] with w_t[k, pi, o] = w[o, k*128 + pi]."""
    o_sh, in_f = w.shape
    wt = w.T.reshape(in_f // P, P, o_sh)  # [k, pi, o]
    return np.ascontiguousarray(wt).astype(dtype)


def make_in_maps(x, weight_mu, weight_rho, bias_mu, bias_rho, weight_eps, bias_eps):
    o_sh = OUT_F // N_CORES
    x_t = _prep_x(np.asarray(x, dtype=np.float32))
    wmu = np.asarray(weight_mu, dtype=np.float32)
    wrho = np.asarray(weight_rho, dtype=np.float32)
    weps = np.asarray(weight_eps, dtype=np.float32)
    bmu = np.asarray(bias_mu, dtype=np.float32).reshape(1, -1)
    brho = np.asarray(bias_rho, dtype=np.float32).reshape(1, -1)
    beps = np.asarray(bias_eps, dtype=np.float32).reshape(1, -1)

    in_maps = []
    for c in range(N_CORES):
        rs = slice(c * o_sh, (c + 1) * o_sh)
        in_maps.append(
            {
                "x_t": x_t,
                "wmu_t": _prep_w(wmu[rs], ml_dtypes.bfloat16),
                "wrho_t": _prep_w(wrho[rs], np.float16),
                "weps_t": _prep_w(weps[rs], ml_dtypes.bfloat16),
                "bias_mu": np.ascontiguousarray(bmu[:, rs]),
                "bias_rho": np.ascontiguousarray(brho[:, rs]),
                "bias_eps": np.ascontiguousarray(beps[:, rs]),
            }
        )
    return in_maps


def kernel(x, weight_mu, weight_rho, bias_mu, bias_rho, weight_eps, bias_eps):
    o_sh = OUT_F // N_CORES
    key = (x.shape, o_sh)
    if key not in _NC_CACHE:
        _NC_CACHE[key] = build_nc(x.shape[0], x.shape[1], o_sh)
    nc = _NC_CACHE[key]

    in_maps = make_in_maps(
        x, weight_mu, weight_rho, bias_mu, bias_rho, weight_eps, bias_eps
    )
    res = run_bass_kernel_spmd(nc, in_maps, core_ids=list(range(N_CORES)))
    return np.concatenate([res.results[c]["y"] for c in range(N_CORES)], axis=1)


# revision 16
# speedup vs baseline: 1.6569x; 1.0162x over previous
"""BayesianLinear (y = x @ (mu + softplus(rho) * eps).T + bias) on 8 TRN2 cores.

Column-parallel sharding: each core owns OUT_F/8 = 512 output features.

Host-side prep is pure layout/precision staging (no reference math):
  - x is cast to bf16 and pre-tiled into the SBUF layout the TensorEngine
    needs for its stationary operand: x_t[bt, pi, po, bi] = x[bt*128+bi,
    po*128+pi], so each 128-row batch tile is one contiguous 1 MiB DMA.
  - weight_mu/rho/eps shards are transposed to [in_f, o_sh] and tiled as
    [in_f/128, 128, o_sh] so W^T can be constructed on-device directly in
    its matmul layout (K on partitions), one contiguous 256 KiB DMA per
    K-block per param.

Device per core:
  1. bias row = bias_mu + softplus(bias_rho) * bias_eps (fp32 ACT+DVE,
     cast bf16). softplus(v) = Ln(1 + Exp(v)) — no Softplus LUT on TRN2;
     Exp and Ln share one ACT table.
  2. For each K-block k (32): load the three param tiles [128, 512] fp32,
     softplus+mul+add on ACT/DVE, write bf16 straight into the resident
     W^T tile [128, 32, 512]. No DRAM bounce, no transpose on device.
  3. For each batch tile bt (64): one linear DMA for x^T [128, 32, 128]
     bf16, 32 accumulating bf16 matmuls into PSUM [128, 512] fp32, bias
     added via a K=1 matmul against a ones row, DVE eviction, DMA out.
  Matmuls for K-block k only depend on W^T block k, so the weight
  construction overlaps the start of the matmul stream.
"""

import numpy as np
import ml_dtypes

import concourse.bacc as bacc
import concourse.mybir as mybir
import concourse.tile as tile
from concourse.bass_utils import run_bass_kernel_spmd

BATCH = 8192
IN_F = 4096
OUT_F = 4096
N_CORES = 8
P = 128

_NC_CACHE = {}


def build_nc(batch=BATCH, in_f=IN_F, o_sh=OUT_F // N_CORES):
    KB = in_f // P  # K-blocks of 128 along the contraction dim
    BT = batch // P  # 128-row output tiles

    nc = bacc.Bacc(
        "TRN2",
        target_bir_lowering=False,
        debug=False,
        enable_asserts=False,
        num_devices=N_CORES,
    )
    bf16 = mybir.dt.bfloat16
    f32 = mybir.dt.float32

    x = nc.declare_dram_parameter("x_t", [BT, P, KB, P], bf16, isOutput=False)
    # mu/eps ship as bf16 (their info is rounded into the bf16 W anyway);
    # rho ships as fp16 — softplus amplifies quantization ~3x, and fp16's
    # 10-bit mantissa keeps that error negligible at half the fp32 bytes.
    f16 = mybir.dt.float16
    wmu = nc.declare_dram_parameter("wmu_t", [KB, P, o_sh], bf16, isOutput=False)
    wrho = nc.declare_dram_parameter("wrho_t", [KB, P, o_sh], f16, isOutput=False)
    weps = nc.declare_dram_parameter("weps_t", [KB, P, o_sh], bf16, isOutput=False)
    bmu = nc.declare_dram_parameter("bias_mu", [1, o_sh], f32, isOutput=False)
    brho = nc.declare_dram_parameter("bias_rho", [1, o_sh], f32, isOutput=False)
    beps = nc.declare_dram_parameter("bias_eps", [1, o_sh], f32, isOutput=False)
    y = nc.declare_dram_parameter("y", [batch, o_sh], f32, isOutput=True)

    act_exp = mybir.ActivationFunctionType.Exp
    act_ln = mybir.ActivationFunctionType.Ln

    with tile.TileContext(nc) as tc:
        with (
            tc.tile_pool(name="const", bufs=1) as const,
            tc.tile_pool(name="wcons", bufs=5) as wcons,
            tc.tile_pool(name="xin", bufs=10) as xin,
            tc.tile_pool(name="yout", bufs=4) as yout,
            tc.tile_pool(name="psum", bufs=8, space="PSUM") as psum_pool,
        ):
            # ---- bias row and ones row for the K=1 bias matmul
            b_mu = const.tile([1, o_sh], f32, tag="b_mu")
            b_rho = const.tile([1, o_sh], f32, tag="b_rho")
            b_eps = const.tile([1, o_sh], f32, tag="b_eps")
            nc.gpsimd.dma_start(out=b_mu[:], in_=bmu[:])
            nc.gpsimd.dma_start(out=b_rho[:], in_=brho[:])
            nc.gpsimd.dma_start(out=b_eps[:], in_=beps[:])
            b_sp = const.tile([1, o_sh], f32, tag="b_sp")
            nc.scalar.activation(b_sp[:], b_rho[:], act_exp)
            nc.scalar.activation(b_sp[:], b_sp[:], act_ln, bias=1.0)
            nc.vector.tensor_mul(out=b_sp[:], in0=b_sp[:], in1=b_eps[:])
            bias_bf = const.tile([1, o_sh], bf16, tag="bias_bf")
            nc.vector.tensor_add(out=bias_bf[:], in0=b_sp[:], in1=b_mu[:])
            ones = const.tile([1, P], bf16, tag="ones")
            nc.vector.memset(ones[:], 1.0)
            # broadcast bias across partitions once: [128, o_sh] = ones.T @ bias
            bias_ps = psum_pool.tile([P, o_sh], f32, tag="ps", name="bias_ps")
            nc.tensor.matmul(bias_ps[:], lhsT=ones[:], rhs=bias_bf[:])
            bias_sb = const.tile([P, o_sh], f32, tag="bias_sb")
            nc.vector.tensor_copy(out=bias_sb[:], in_=bias_ps[:])

            # ---- W^T constructed in place, one K-block at a time.
            # Param DMAs ride the GPSIMD SWDGE queue: the sync queue stays
            # dedicated to x-tile loads and the ACT engine to exp/ln.
            WT = const.tile([P, KB, o_sh], bf16, tag="WT")
            for k in range(KB):
                mu_t = wcons.tile([P, o_sh], bf16, tag="mu")
                rho_t = wcons.tile([P, o_sh], f16, tag="rho")
                eps_t = wcons.tile([P, o_sh], bf16, tag="eps")
                nc.gpsimd.dma_start(out=mu_t[:], in_=wmu[k])
                nc.gpsimd.dma_start(out=rho_t[:], in_=wrho[k])
                nc.gpsimd.dma_start(out=eps_t[:], in_=weps[k])
                sp_t = wcons.tile([P, o_sh], f32, tag="sp")
                nc.scalar.activation(sp_t[:], rho_t[:], act_exp)
                nc.scalar.activation(sp_t[:], sp_t[:], act_ln, bias=1.0)
                nc.vector.tensor_mul(out=sp_t[:], in0=sp_t[:], in1=eps_t[:])
                nc.vector.tensor_add(out=WT[:, k, :], in0=sp_t[:], in1=mu_t[:])

            def body_tail(ps, bt):
                y_sb = yout.tile([P, o_sh], f32, tag="y_sb")
                nc.vector.tensor_add(out=y_sb[:], in0=ps[:], in1=bias_sb[:])
                nc.sync.dma_start(out=y[bt * P : (bt + 1) * P, :], in_=y_sb[:])

            # ---- first GROUP tiles run k-interleaved across PSUM banks so
            # the PE consumes W^T blocks no faster than construction makes
            # them — the weight-construction latency hides under matmuls.
            GROUP = min(8, BT)
            xts = []
            pss = []
            for bt in range(GROUP):
                xT = xin.tile([P, KB, P], bf16, tag="xT", name=f"xT_g{bt}")
                xts.append(xT)
                ps = psum_pool.tile([P, o_sh], f32, tag="ps", name=f"ps_g{bt}")
                pss.append(ps)
            # chunk-major strip loads: the first K-quarter of every strip
            # lands before any second quarter, so the k=0 matmul batch isn't
            # gated on the last strip's full 1 MiB transfer (Tile tracks
            # deps at AP-range granularity).
            CH = 4 if KB % 4 == 0 else 1
            for c in range(CH):
                ks = slice(c * (KB // CH), (c + 1) * (KB // CH))
                for i in range(GROUP):
                    nc.sync.dma_start(out=xts[i][:, ks, :], in_=x[i, :, ks, :])
            for k in range(KB):
                for i in range(GROUP):
                    nc.tensor.matmul(
                        pss[i][:],
                        lhsT=xts[i][:, k, :],
                        rhs=WT[:, k, :],
                        start=(k == 0),
                        stop=(k == KB - 1),
                    )
            for i in range(GROUP):
                body_tail(pss[i], i)

            # ---- remaining tiles stream one PSUM bank each
            for bt in range(GROUP, BT):
                xT = xin.tile([P, KB, P], bf16, tag="xT")
                nc.sync.dma_start(out=xT[:], in_=x[bt])
                ps = psum_pool.tile([P, o_sh], f32, tag="ps")
                for k in range(KB):
                    nc.tensor.matmul(
                        ps[:],
                        lhsT=xT[:, k, :],
                        rhs=WT[:, k, :],
                        start=(k == 0),
                        stop=(k == KB - 1),
                    )
                body_tail(ps, bt)

    # Skip bacc's pre-placed InstLoadActFuncSet: on large graphs walrus's
    # parallel-pass fork can separate the hoisted load from its activations
    # ("No Act func set exist for this instruction"); walrus's own lower_act
    # placement handles forked subgraphs correctly.
    nc.insert_act_table_loads = lambda: None
    nc.compile()
    return nc


def _prep_x(x):
    """[batch, in_f] fp32 -> bf16 tiled [BT, 128, KB, 128] with
    x_t[bt, pi, po, bi] = x[bt*128 + bi, po*128 + pi]."""
    batch, in_f = x.shape
    xb = x.astype(ml_dtypes.bfloat16)
    xb = xb.reshape(batch // P, P, in_f // P, P)  # [bt, bi, po, pi]
    return np.ascontiguousarray(xb.transpose(0, 3, 2, 1))  # [bt, pi, po, bi]


def _prep_w(w, dtype=np.float32):
    """[o_sh, in_f] -> tiled [KB, 128, o_sh] with w_t[k, pi, o] = w[o, k*128 + pi]."""
    o_sh, in_f = w.shape
    wt = w.T.reshape(in_f // P, P, o_sh)  # [k, pi, o]
    return np.ascontiguousarray(wt).astype(dtype)


def make_in_maps(x, weight_mu, weight_rho, bias_mu, bias_rho, weight_eps, bias_eps):
    o_sh = OUT_F // N_CORES
    x_t = _prep_x(np.asarray(x, dtype=np.float32))
    wmu = np.asarray(weight_mu, dtype=np.float32)
    wrho = np.asarray(weight_rho, dtype=np.float32)
    weps = np.asarray(weight_eps, dtype=np.float32)
    bmu = np.asarray(bias_mu, dtype=np.float32).reshape(1, -1)
    brho = np.asarray(bias_rho, dtype=np.float32).reshape(1, -1)
    beps = np.asarray(bias_eps, dtype=np.float32).reshape(1, -1)

    in_maps = []
    for c in range(N_CORES):
        rs = slice(c * o_sh, (c + 1) * o_sh)
        in_maps.append(
            {
                "x_t": x_t,
                "wmu_t": _prep_w(wmu[rs], ml_dtypes.bfloat16),
                "wrho_t": _prep_w(wrho[rs], np.float16),
                "weps_t": _prep_w(weps[rs], ml_dtypes.bfloat16),
                "bias_mu": np.ascontiguousarray(bmu[:, rs]),
                "bias_rho": np.ascontiguousarray(brho[:, rs]),
                "bias_eps": np.ascontiguousarray(beps[:, rs]),
            }
        )
    return in_maps


def kernel(x, weight_mu, weight_rho, bias_mu, bias_rho, weight_eps, bias_eps):
    o_sh = OUT_F // N_CORES
    key = (x.shape, o_sh)
    if key not in _NC_CACHE:
        _NC_CACHE[key] = build_nc(x.shape[0], x.shape[1], o_sh)
    nc = _NC_CACHE[key]

    in_maps = make_in_maps(
        x, weight_mu, weight_rho, bias_mu, bias_rho, weight_eps, bias_eps
    )
    res = run_bass_kernel_spmd(nc, in_maps, core_ids=list(range(N_CORES)))
    return np.concatenate([res.results[c]["y"] for c in range(N_CORES)], axis=1)
